# revision 22
# baseline (speedup 1.0000x reference)
"""BiLevelRoutingAttention (spiking) Trainium2 kernel.

Sharding: one (t, b) pair per core (T=4 x B=2 = 8 cores). All windows of a
(t,b) live on one core, so the routed-KV gather is local. The only cross-core
data is the routing region mean (over t and s), realized as a tiny [128,64]
AllReduce among the 4 cores sharing each b (replica groups {0..3}, {4..7};
core_id = b*4 + t).

Key optimizations over the straightforward version:
  - qkv / gram / proj matmuls run in fp8 (e4m3) with DoubleRow perf mode
    (contraction 256 in one pass, 2x PE throughput). Spikes are {0,1} so the
    gram path is numerically exact; qkv quantization error (~2%) is ~6 sigma
    away from the spike threshold.
  - per-output-channel LIF thresholds th = 2 - b are folded into the weights
    (w' = 64*w/th, th > 0 always), making the spike threshold the constant 64.
    This lets the Scalar engine compute spikes via a saturated Sigmoid with an
    immediate-style bias, splitting spike work across Vector and Scalar.
  - the divide-by-zero guard eps is accumulated into the den PSUM tile by a
    1-partition matmul (eps_row^T @ ones), so normalization is just
    reciprocal_approx_fast + multiply on Vector, batched per window pair.
  - combine (sel^T @ gram rows) uses a block-diagonal [128,128] sel^T
    stationary: 32 full-width matmuls instead of 96 32-partition ones.
  - routing/gather layouts are aligned so every DMA run is >=129 elements.
"""

import numpy as np
import ml_dtypes

T, B, Lt, Lh, Lw, C = 4, 2, 8, 32, 32, 256
WT, WH, WW = 2, 4, 4
LT, LH, LW = Lt // WT, Lh // WH, Lw // WW  # 4, 8, 8
W = WT * WH * WW        # 32 windows
S = LT * LH * LW        # 256 tokens per window
NTOK = W * S            # 8192
H, D = 8, 32
TOPK = 4
NCORES = 8
GROUPS = [[0, 1, 2, 3], [4, 5, 6, 7]]
THR = 64.0              # folded spike threshold
SGS = 1.0e4             # sigmoid saturation scale
PSCALE = 32.0           # proj weight scale (fp8 range)
BF16 = ml_dtypes.bfloat16
FP8 = ml_dtypes.float8_e4m3

_CACHE = {}


def build_kernel():
    from concourse import bacc
    import concourse.mybir as mybir
    import concourse.tile as tile
    from concourse.tile_rust import add_dep_helper
    from concourse.masks import make_identity

    f8 = mybir.dt.float8e4
    bf = mybir.dt.bfloat16
    f32 = mybir.dt.float32
    DR = mybir.MatmulPerfMode.DoubleRow
    ACT = mybir.ActivationFunctionType
    ALU = mybir.AluOpType

    nc = bacc.Bacc("TRN2", target_bir_lowering=False, debug=False,
                   num_devices=NCORES)

    xT = nc.dram_tensor("xT", [2, 128, NTOK], f8, kind="ExternalInput")
    wq = nc.dram_tensor("wq", [128, 2, 2, 128], f8, kind="ExternalInput")
    wkv = nc.dram_tensor("wkv", [128, 2, 512], f8, kind="ExternalInput")
    wproj = nc.dram_tensor("wproj", [128, 2, 2, 128], f8, kind="ExternalInput")
    bproj = nc.dram_tensor("bproj", [128, 2], f32, kind="ExternalInput")
    bmask = nc.dram_tensor("bmask", [128, 129], bf, kind="ExternalInput")
    growmask = nc.dram_tensor("growmask", [128, 4, 129], bf, kind="ExternalInput")
    outT = nc.dram_tensor("outT", [2, 128, NTOK], f32, kind="ExternalOutput")
    sel_dbg = nc.dram_tensor("sel_dbg", [32, 32], f32, kind="ExternalOutput")

    cc_in = nc.dram_tensor("cc_in", [128, 64], f32)
    cc_out = nc.dram_tensor("cc_out", [128, 64], f32)

    with tile.TileContext(nc) as tc:
        with (
            tc.tile_pool(name="big", bufs=2) as big_pool,
            tc.tile_pool(name="persist", bufs=1) as pp,
            tc.tile_pool(name="kvs", bufs=4) as kv_pool,
            tc.tile_pool(name="gsb", bufs=4) as gsb_pool,
            tc.tile_pool(name="grow", bufs=4) as grow_pool,
            tc.tile_pool(name="small", bufs=2) as sm_pool,
            tc.tile_pool(name="dsc", bufs=4) as dsc_pool,
            tc.tile_pool(name="outp", bufs=4) as out_pool,
            tc.tile_pool(name="mm512", bufs=5, space="PSUM") as mm512,
            tc.tile_pool(name="misc", bufs=3, space="PSUM") as misc,
            tc.tile_pool(name="dram", bufs=1, space="DRAM") as dram_pool,
        ):
            # ---- load x + weights; first block's needs dispatched first ----
            xsb = big_pool.tile([128, 2, NTOK], f8, tag="bigbuf", bufs=1)
            for c in range(2):
                nc.sync.dma_start(xsb[:, c, 0:512], xT[c, :, 0:512])
            wq_sb = pp.tile([128, 2, 2, 128], f8)
            nc.sync.dma_start(wq_sb[:], wq[:])
            wkv_sb = pp.tile([128, 2, 512], f8)
            nc.sync.dma_start(wkv_sb[:], wkv[:])
            for c in range(2):
                nc.sync.dma_start(xsb[:, c, 512:2048], xT[c, :, 512:2048])
            for p in range(1, 4):
                sl = slice(p * 2048, (p + 1) * 2048)
                for c in range(2):
                    nc.sync.dma_start(xsb[:, c, sl], xT[c, :, sl])
            wproj_sb = pp.tile([128, 2, 2, 128], f8)
            nc.sync.dma_start(wproj_sb[:], wproj[:])
            bproj_sb = pp.tile([128, 2], f32)
            nc.sync.dma_start(bproj_sb[:], bproj[:])
            bmask_sb = pp.tile([128, 129], bf)
            nc.sync.dma_start(bmask_sb[:], bmask[:])
            growmask_sb = pp.tile([128, 4, 129], bf)
            nc.sync.dma_start(growmask_sb[:], growmask[:])
            id32 = pp.tile([32, 32], f32)
            make_identity(nc, id32[:])
            sgbias = pp.tile([128, 1], f32)
            nc.vector.memset(sgbias[:], -THR * SGS)
            epsmat = pp.tile([128, 128], bf)
            nc.vector.memset(epsmat[:], 1e-6 / 128)
            ones1 = pp.tile([128, 512], bf)
            nc.vector.memset(ones1[:], 1.0)
            selbd = pp.tile([128, 128], bf)
            nc.gpsimd.memset(selbd[:], 0.0)

            # kvt spike tiles: [tok128, chunk2, k(256)|v0(128)|1|v1(128)|1]
            # ones columns initialized once per pool buffer; spikes never
            # touch cols 384/513 so they survive buffer reuse.
            kvt_bufs = []
            for i in range(4):
                # row pitch 528 (%16==0) required by DoubleRow k-tile step
                t = kv_pool.tile([128, 2, 528], f8, tag="kvt")
                nc.gpsimd.memset(t[:, :, 384:385], 1.0)
                nc.gpsimd.memset(t[:, :, 513:514], 1.0)
                kvt_bufs.append(t)

            # ---- qkv + spikes + per-window Grams (fp8 DoubleRow) ----
            qsb = big_pool.tile([128, 2, NTOK], bf, tag="qsb", bufs=1)
            gram_dram = dram_pool.tile([32, 128, 258], bf)
            gflat = gram_dram[:].rearrange("w p e -> w (p e)")
            kvr_dram = dram_pool.tile([128, 32, 258], bf)
            kvread = pp.tile([128, 32, 2, 129], bf)
            dexp = pp.tile([128, 32, 2, 128], bf)
            cc = None
            for blk in range(16):
                if blk == 3:
                    # region sums: c=0 on Vector (single reduce), c=1 on the
                    # otherwise-idle GpSimd as a bf16 halving add-tree.
                    region = pp.tile([128, 2, 32], f32)
                    nc.vector.reduce_sum(
                        region[:, 0, :],
                        xsb[:, 0, :].rearrange("p (w s) -> p w s", s=S),
                        axis=mybir.AxisListType.X,
                    )
                    rtree = pp.tile([128, 32, 128], bf)
                    xv = xsb[:, 1, :].rearrange("p (w s) -> p w s", s=S)
                    nc.gpsimd.tensor_tensor(rtree[:], xv[:, :, 0:128],
                                            xv[:, :, 128:256], op=ALU.add)
                    hw = 64
                    while hw >= 1:
                        dst = region[:, 1, :] if hw == 1 else                             rtree[:, :, 0:hw]
                        nc.gpsimd.tensor_tensor(
                            dst, rtree[:, :, 0:hw],
                            rtree[:, :, hw:2 * hw], op=ALU.add)
                        hw //= 2
                    st = nc.gpsimd.dma_start(
                        cc_in[:], region[:].rearrange("p a w -> p (a w)"))
                    cc = nc.gpsimd.collective_compute(
                        "AllReduce", ALU.add, replica_groups=GROUPS,
                        ins=[cc_in[:]], outs=[cc_out[:]],
                    )
                    add_dep_helper(cc.ins, st.ins,
                                   reason="region stored before collective")
                tsl = slice(blk * 512, (blk + 1) * 512)
                for qc in range(2):
                    qp = mm512.tile([128, 512], f32, tag="mm512")
                    nc.tensor.matmul(qp[:], wq_sb[:, :, qc, :], xsb[:, :, tsl],
                                     start=True, stop=True, perf_mode=DR)
                    nc.vector.tensor_scalar(qsb[:, qc, tsl], qp[:],
                                            THR, None, op0=ALU.is_ge)
                for tci in range(4):
                    tcg = blk * 4 + tci
                    w, ch = tcg // 2, tcg % 2
                    kvt = kvt_bufs[w % 4]
                    ksl = slice(tcg * 128, (tcg + 1) * 128)
                    kvp = mm512.tile([128, 512], f32, tag="mm512")
                    nc.tensor.matmul(kvp[:], xsb[:, :, ksl], wkv_sb[:],
                                     start=True, stop=True, perf_mode=DR)
                    if tcg % 5 < 2:
                        nc.vector.tensor_scalar(kvt[:, ch, 0:384], kvp[:, 0:384],
                                                THR, None, op0=ALU.is_ge)
                        nc.vector.tensor_scalar(kvt[:, ch, 385:513], kvp[:, 384:512],
                                                THR, None, op0=ALU.is_ge)
                    else:
                        nc.scalar.activation(kvt[:, ch, 0:384], kvp[:, 0:384],
                                             ACT.Sigmoid, bias=sgbias[:], scale=SGS)
                        nc.scalar.activation(kvt[:, ch, 385:513], kvp[:, 384:512],
                                             ACT.Sigmoid, bias=sgbias[:], scale=SGS)
                gsb = gsb_pool.tile([128, 2, 2, 129], bf, tag="gsb")
                for wi, w in enumerate((blk * 2, blk * 2 + 1)):
                    kvt = kvt_bufs[w % 4]
                    for c in range(2):
                        gp = misc.tile([128, 129], f32, tag="misc")
                        nc.tensor.matmul(gp[:], kvt[:, :, 128 * c:128 * (c + 1)],
                                         kvt[:, :, 256 + 129 * c:385 + 129 * c],
                                         start=True, stop=True, perf_mode=DR)
                        if c == 0:
                            nc.vector.tensor_tensor(gsb[:, wi, c, :], gp[:],
                                                    bmask_sb[:], op=ALU.mult)
                        else:
                            nc.scalar.activation(gsb[:, wi, c, :], gp[:], ACT.Copy)
                nc.sync.dma_start(
                    gram_dram[2 * blk:2 * blk + 2].rearrange("w p e -> p w e"),
                    gsb[:].rearrange("p w c e -> p w (c e)"))

            # ---- scores -> top-4 selection matrix, block-diag sel^T ----
            grows = []
            for qtr in range(2):
                g = grow_pool.tile([128, 2064], bf, tag="grow", bufs=3)
                nc.sync.dma_start(
                    g[:], gflat[:, qtr * 8256:(qtr + 1) * 8256].rearrange(
                        "w (j e) -> j w e", j=4))
                grows.append(g)
            xs_sb = pp.tile([128, 2, 32], f32)
            ld = nc.gpsimd.dma_start(
                xs_sb[:], cc_out[:].rearrange("p (a w) -> p a w", w=32))
            add_dep_helper(ld.ins, cc.ins, reason="collective before readback")
            scp = misc.tile([32, 32], f32, tag="misc")
            for c in range(2):
                nc.tensor.matmul(scp[:], xs_sb[:, c, :], xs_sb[:, c, :],
                                 start=(c == 0), stop=(c == 1))
            shifted = sm_pool.tile([32, 32], f32, tag="shifted")
            nc.vector.tensor_scalar(shifted[:], scp[:], 1e6, None, op0=ALU.add)
            mx8 = sm_pool.tile([32, 8], f32, tag="mx8")
            nc.vector.max(mx8[:], shifted[:])
            nc.vector.memset(mx8[:, TOPK:], 0.0)
            zapped = sm_pool.tile([32, 32], f32, tag="zapped")
            nc.vector.match_replace(out=zapped[:], in_to_replace=mx8[:],
                                    in_values=shifted[:], imm_value=0.0)
            selb = sm_pool.tile([32, 32], f32, tag="selb")
            nc.vector.tensor_tensor(selb[:], shifted[:], zapped[:], op=ALU.is_gt)
            selT_ps = misc.tile([32, 32], f32, tag="misc")
            nc.tensor.transpose(selT_ps[:], selb[:], id32[:])
            selT = sm_pool.tile([32, 32], bf, tag="selT")
            nc.vector.tensor_copy(selT[:], selT_ps[:])
            for j in range(4):
                nc.sync.dma_start(selbd[32 * j:32 * (j + 1), 32 * j:32 * (j + 1)],
                                  selT[:])

            # ---- combine kvr[w] = sum_j sel[w,j] G[j] ----
            # gflat rows: [w, (p c e)]; quarter q covers p in [32q, 32q+32);
            # j-group of 32 partitions holds 8 p's (2064 cols). The head
            # block-diagonal mask is constant along w, so it is applied to
            # grow on GpSimd (column-wise) before the matmul. The w->p
            # transpose is a direct SBUF->SBUF DMA into kvread (258B runs).
            for qtr in range(4):
                grow = grows[qtr]
                if qtr + 2 < 4:
                    g = grow_pool.tile([128, 2064], bf, tag="grow", bufs=3)
                    nc.sync.dma_start(
                        g[:], gflat[:, (qtr + 2) * 8256:(qtr + 3) * 8256].rearrange(
                            "w (j e) -> j w e", j=4))
                    grows.append(g)
                gv = grow[:].rearrange("q (p c e) -> q p c e", p=8, c=2)
                nc.gpsimd.tensor_tensor(
                    gv[:, :, 1, :], gv[:, :, 1, :],
                    growmask_sb[:, qtr, None, :].to_broadcast([128, 8, 129]),
                    op=ALU.mult)
                kvout = grow_pool.tile([128, 2064], bf, tag="kvout", bufs=2)
                for ch in range(8):
                    csl = slice(ch * 258, (ch + 1) * 258)
                    cp = misc.tile([128, 258], f32, tag="misc")
                    nc.tensor.matmul(cp[:], selbd[:], grow[:, csl],
                                     start=True, stop=True)
                    nc.scalar.activation(kvout[:, csl], cp[:], ACT.Copy)
                for j in range(4):
                    pb = 32 * qtr + 8 * j
                    nc.sync.dma_start(
                        kvr_dram[pb:pb + 8].rearrange("p w e -> w p e"),
                        kvout[32 * j:32 * (j + 1), :].rearrange(
                            "w (p e) -> w p e", p=8))
                psl = slice(32 * qtr, 32 * (qtr + 1))
                nc.sync.dma_start(
                    kvread[psl, :, :, :].rearrange("p w c e -> p w (c e)"),
                    kvr_dram[psl])
            # den stationary: ksum column replicated across the head block
            for c in range(2):
                eng = nc.vector if c == 0 else nc.gpsimd
                eng.tensor_tensor(
                    dexp[:, :, c, :],
                    kvread[:, :, c, 128:129].to_broadcast([128, 32, 128]),
                    bmask_sb[:, None, 0:128].to_broadcast([128, 32, 128]),
                    op=ALU.mult)

            # ---- attention, batched per window pair ----
            attn_nb = big_pool.tile([128, 2, NTOK], f8, tag="bigbuf", bufs=1)

            def attn_pair(blk, c):
                tsl = slice(blk * 512, (blk + 1) * 512)
                ap = mm512.tile([128, 512], f32, tag="mm512")
                dp = mm512.tile([128, 512], f32, tag="mm512")
                nc.tensor.matmul(dp[:], epsmat[:], ones1[:],
                                 start=True, stop=False, skip_group_check=True)
                for i, w in enumerate((blk * 2, blk * 2 + 1)):
                    osl = slice(i * 256, (i + 1) * 256)
                    wsl = slice(w * 256, (w + 1) * 256)
                    nc.tensor.matmul(ap[:, osl], kvread[:, w, c, 0:128],
                                     qsb[:, c, wsl], start=True, stop=True)
                    nc.tensor.matmul(dp[:, osl], dexp[:, w, c, :],
                                     qsb[:, c, wsl], start=False, stop=True,
                                     skip_group_check=True)
                dsc = dsc_pool.tile([128, 512], f32, tag="dsc")
                nc.vector.reciprocal_approx_fast(out=dsc[:], in_=dp[:])
                if blk % 2 == 0:
                    nc.vector.tensor_tensor(attn_nb[:, c, tsl], ap[:], dsc[:],
                                            op=ALU.mult)
                else:
                    asb = dsc_pool.tile([128, 512], f32, tag="asb", bufs=3)
                    nc.scalar.activation(asb[:], ap[:], ACT.Copy)
                    nc.gpsimd.tensor_tensor(attn_nb[:, c, tsl], asb[:], dsc[:],
                                            op=ALU.mult)

            def proj_blk(blk):
                tsl = slice(blk * 512, (blk + 1) * 512)
                for pc in range(2):
                    pjp = misc.tile([128, 512], f32, tag="misc")
                    nc.tensor.matmul(pjp[:], wproj_sb[:, :, pc, :],
                                     attn_nb[:, :, tsl],
                                     start=True, stop=True, perf_mode=DR)
                    osb = out_pool.tile([128, 512], f32, tag="osb")
                    nc.scalar.activation(osb[:], pjp[:], ACT.Identity,
                                         bias=bproj_sb[:, pc:pc + 1],
                                         scale=1.0 / PSCALE)
                    nc.sync.dma_start(outT[pc, :, tsl], osb[:])

            for blk in range(16):
                attn_pair(blk, 0)
            for blk in range(16):
                attn_pair(blk, 1)
                if blk >= 3:
                    proj_blk(blk - 3)
            for blk in range(13, 16):
                proj_blk(blk)
            nc.sync.dma_start(sel_dbg[:], selb[:])

    nc.compile()
    return nc


def _prep_shared(w_qkv, b_qkv, w_proj, b_proj):
    th = 2.0 - b_qkv                      # > 0 always (b ~ 0.02*randn)
    wf = w_qkv * (THR / th)[None, :]      # fold thresholds: spike at x@w' >= 64
    wq_a = wf[:, 0:256].reshape(2, 128, 2, 128).transpose(1, 0, 2, 3)
    wkv_a = wf[:, 256:768].reshape(2, 128, 512).transpose(1, 0, 2)
    wproj_a = (w_proj * PSCALE).reshape(2, 128, 2, 128).transpose(1, 0, 2, 3)
    bproj_a = b_proj.reshape(2, 128).T
    i = np.arange(128)[:, None]
    j = np.arange(129)[None, :]
    bmask_a = ((i // 32) == (j // 32)) | (j == 128)
    e = np.arange(129)
    q = np.arange(4)[:, None]
    gm_row = ((e[None, :] // 32) == q) | (e[None, :] == 128)   # [4, 129]
    gm = np.broadcast_to(gm_row[None, :, :], (128, 4, 129))
    return {
        "wq": np.ascontiguousarray(wq_a).astype(FP8),
        "wkv": np.ascontiguousarray(wkv_a).astype(FP8),
        "wproj": np.ascontiguousarray(wproj_a).astype(FP8),
        "bproj": np.ascontiguousarray(bproj_a).astype(np.float32),
        "bmask": bmask_a.astype(BF16),
        "growmask": np.ascontiguousarray(gm).astype(BF16),
    }


def window_partition(x):
    """[T,B,Lt,Lh,Lw,C] -> [T,B,NTOK,C] with tokens in (w, s) order."""
    Tb, Bb = x.shape[0], x.shape[1]
    xw = x.reshape(Tb, Bb, WT, LT, WH, LH, WW, LW, C)
    xw = xw.transpose(0, 1, 2, 4, 6, 3, 5, 7, 8)
    return np.ascontiguousarray(xw).reshape(Tb, Bb, NTOK, C)


def window_reverse(o):
    """[NTOK, C] -> [Lt, Lh, Lw, C]."""
    o = o.reshape(WT, WH, WW, LT, LH, LW, C)
    o = o.transpose(0, 3, 1, 4, 2, 5, 6)
    return np.ascontiguousarray(o).reshape(Lt, Lh, Lw, C)


def make_in_maps(x, w_qkv, b_qkv, w_proj, b_proj):
    shared = _prep_shared(w_qkv, b_qkv, w_proj, b_proj)
    xw = window_partition(x)
    in_maps = []
    for core in range(NCORES):
        b, t = core // 4, core % 4
        xt = np.ascontiguousarray(xw[t, b].T).astype(FP8)  # [C, NTOK]
        in_maps.append({**shared, "xT": xt.reshape(2, 128, NTOK)})
    return in_maps


def run_kernel_spmd(nc, in_maps, **kwargs):
    from concourse.bass_utils import run_bass_kernel_spmd
    return run_bass_kernel_spmd(nc, in_maps, core_ids=list(range(NCORES)), **kwargs)


def kernel(x, w_qkv, b_qkv, w_proj, b_proj):
    x = np.asarray(x, dtype=np.float32)
    w_qkv = np.asarray(w_qkv, dtype=np.float32)
    b_qkv = np.asarray(b_qkv, dtype=np.float32)
    w_proj = np.asarray(w_proj, dtype=np.float32)
    b_proj = np.asarray(b_proj, dtype=np.float32)

    if "nc" not in _CACHE:
        _CACHE["nc"] = build_kernel()
    nc = _CACHE["nc"]

    res = run_kernel_spmd(nc, make_in_maps(x, w_qkv, b_qkv, w_proj, b_proj))

    out = np.empty((T, B, Lt, Lh, Lw, C), dtype=np.float32)
    for core in range(NCORES):
        b, t = core // 4, core % 4
        oT = res.results[core]["outT"].reshape(256, NTOK)
        out[t, b] = window_reverse(np.ascontiguousarray(oT.T))
    return out
